# revision 1
# baseline (speedup 1.0000x reference)
"""GraphSage (2-layer, mean aggr) on 8 trn2 NeuronCores.

Scheme (dst-sharded edge-parallel):
  - Nodes padded to 50176 = 8 * 6272; core c owns dst nodes [c*6272, (c+1)*6272).
  - Each core processes exactly the edges targeting its dst shard (~100k).
  - Edges bucketed by 64-node dst tile; within tile split by src < 32768
    (dma_gather indices are int16), each (tile, half) padded to a multiple
    of 128 edges (pad: src=0, dst_local=-1 -> zero one-hot weight).
  - Gather x[src] rows (256B) with dma_gather; scatter-add into PSUM via
    one-hot matmuls (lhsT = onehot [128e, 64n], rhs = msg [128e, 64f]).
  - Mean normalization via host-precomputed 1/max(deg,1) (ScalarE scale).
  - agg tiles transposed on TensorE; GEMMs keep weights stationary:
    hT = W1l.T @ aggT + W1r.T @ xT (+b1 per-partition), ELU composed as
    relu(z) + min(exp(z)-1, 0).
  - h written node-major to DRAM, AllGather across 8 cores, layer 2 gathers
    from h_full; output returned feature-major and transposed on host.
"""

import sys

sys.path.insert(0, "/opt/trn_rl_repo")

import numpy as np

import concourse.bacc as bacc
import concourse.mybir as mybir
import concourse.tile as tile
from concourse.bass_utils import run_bass_kernel_spmd

N, E, D, H, O = 50000, 800000, 64, 64, 16
NCORES = 8
NSH = 6272                  # dst nodes per core
NPAD = NSH * NCORES         # 50176
TD = 64                     # dst tile size (one-hot columns)
NT = NSH // TD              # 98 dst tiles per core
CH = 128                    # edges per matmul chunk (contraction dim)
HALFN = 32768               # int16 gather base split
TB = 8                      # dst tiles per gather batch
SUB = 32                    # chunks per one-hot build op

f32 = mybir.dt.float32
i16 = mybir.dt.int16


def _preprocess(edge_index):
    src = np.asarray(edge_index[0], dtype=np.int64)
    dst = np.asarray(edge_index[1], dtype=np.int64)
    core = dst // NSH
    ldst = dst - core * NSH
    tid = ldst // TD
    tloc = (ldst % TD).astype(np.float32)
    half = (src >= HALFN).astype(np.int64)

    key = (core * NT + tid) * 2 + half
    order = np.argsort(key, kind="stable")
    src_s = src[order]
    tloc_s = tloc[order]
    counts = np.bincount(key[order], minlength=NCORES * NT * 2).reshape(
        NCORES, NT, 2
    )
    starts = np.zeros(NCORES * NT * 2 + 1, dtype=np.int64)
    np.cumsum(counts.reshape(-1), out=starts[1:])

    cmax = counts.max(axis=0)  # [NT, 2]
    nch_lo = np.maximum((cmax[:, 0] + CH - 1) // CH, 1).astype(np.int64)
    nch_hi = ((cmax[:, 1] + CH - 1) // CH).astype(np.int64)

    # shared slot/chunk structure, batch-major, lo section then hi section
    batches = []
    slot = 0
    gchunk = 0
    for b0 in range(0, NT, TB):
        tids = list(range(b0, min(b0 + TB, NT)))
        lo_cols, hi_cols = [], []
        col = 0
        for t in tids:
            lo_cols.append((t, col, int(nch_lo[t])))
            col += int(nch_lo[t])
        n_lo = col
        for t in tids:
            hi_cols.append((t, col, int(nch_hi[t])))
            col += int(nch_hi[t])
        n_all = col
        batches.append(
            dict(
                tids=tids,
                lo_cols=lo_cols,
                hi_cols=hi_cols,
                n_lo=n_lo,
                n_hi=n_all - n_lo,
                n_all=n_all,
                slot0=slot,
                gchunk0=gchunk,
            )
        )
        slot += n_all * CH
        gchunk += n_all
    S = slot           # total slots per core
    NCHT = gchunk      # total chunks per core
    assert S % 128 == 0 and S // 16 * 16 == S

    # per-core arrays
    idx_wraps, dl_wraps, recs, xTs = [], [], [], []
    for k in range(NCORES):
        idx_flat = np.zeros(S, dtype=np.int16)
        dl_flat = np.full(S, -1.0, dtype=np.float32)
        for b in batches:
            for (t, c0, nch), h in (
                [(x, 0) for x in b["lo_cols"]] + [(x, 1) for x in b["hi_cols"]]
            ):
                if nch == 0:
                    continue
                ki = (k * NT + t) * 2 + h
                cnt = int(counts[k, t, h])
                s0 = b["slot0"] + c0 * CH
                e0 = int(starts[ki])
                vals = src_s[e0 : e0 + cnt] - h * HALFN
                idx_flat[s0 : s0 + cnt] = vals.astype(np.int16)
                dl_flat[s0 : s0 + cnt] = tloc_s[e0 : e0 + cnt]
        # indices live in 16 partitions, replicated for each of the 8 Q7 cores
        iw = np.ascontiguousarray(
            np.tile(idx_flat.reshape(S // 16, 16).T, (8, 1))
        )
        idx_wraps.append(iw)
        dl_wraps.append(np.ascontiguousarray(dl_flat.reshape(S // CH, CH).T))

        lo = k * NSH
        hi = min((k + 1) * NSH, N)
        deg = np.bincount(ldst[core == k], minlength=NSH).astype(np.float32)
        rec = 1.0 / np.maximum(deg, 1.0)
        recs.append(np.ascontiguousarray(rec.reshape(NT, TD).T))  # [64, NT]
        xt = np.zeros((D, NSH), dtype=np.float32)
        xTs.append(xt)  # filled later with x data by caller
        xTs[-1] = (lo, hi)

    struct = dict(batches=batches, S=S, NCHT=NCHT)
    percore = dict(idx=idx_wraps, dl=dl_wraps, rec=recs, shards=xTs)
    return struct, percore


DEBUG_L1_ONLY = False
DEBUG_HFULL = False


def _build(nc, struct):
    S = struct["S"]
    NCHT = struct["NCHT"]
    batches = struct["batches"]

    x_d = nc.dram_tensor("x", [N, D], f32, kind="ExternalInput")
    xT_d = nc.dram_tensor("xT", [D, NSH], f32, kind="ExternalInput")
    idx_d = nc.dram_tensor("idx", [128, S // 16], i16, kind="ExternalInput")
    dl_d = nc.dram_tensor("dstloc", [128, NCHT], f32, kind="ExternalInput")
    rec_d = nc.dram_tensor("rec", [TD, NT], f32, kind="ExternalInput")
    w1l_d = nc.dram_tensor("w1l", [D, H], f32, kind="ExternalInput")
    w1r_d = nc.dram_tensor("w1r", [D, H], f32, kind="ExternalInput")
    b1_d = nc.dram_tensor("b1", [H, 1], f32, kind="ExternalInput")
    w2l_d = nc.dram_tensor("w2l", [H, O], f32, kind="ExternalInput")
    w2r_d = nc.dram_tensor("w2r", [H, O], f32, kind="ExternalInput")
    b2_d = nc.dram_tensor("b2", [O, 1], f32, kind="ExternalInput")
    iota_d = nc.dram_tensor("iota", [128, TD], f32, kind="ExternalInput")
    ident_d = nc.dram_tensor("ident", [TD, TD], f32, kind="ExternalInput")
    outT_d = nc.dram_tensor("outT", [O, NSH], f32, kind="ExternalOutput")
    hout_d = (
        nc.dram_tensor("hout", [NSH, D], f32, kind="ExternalOutput")
        if DEBUG_L1_ONLY
        else None
    )
    hfull_d = (
        nc.dram_tensor("hfull", [NPAD, D], f32, kind="ExternalOutput")
        if DEBUG_HFULL
        else None
    )

    with tile.TileContext(nc, num_cores=NCORES) as tc:
        with (
            tc.tile_pool(name="const", bufs=1) as cpool,
            tc.tile_pool(name="msg", bufs=2) as mpool,
            tc.tile_pool(name="oh", bufs=3) as ohpool,
            tc.tile_pool(name="agg", bufs=4) as aggpool,
            tc.tile_pool(name="aggT", bufs=49) as aggTpool,
            tc.tile_pool(name="hT", bufs=49) as hTpool,
            tc.tile_pool(name="small", bufs=4) as spool,
            tc.tile_pool(name="ps_sc", bufs=3, space="PSUM") as ps_sc,
            tc.tile_pool(name="ps_tr", bufs=2, space="PSUM") as ps_tr,
            tc.tile_pool(name="ps_tr2", bufs=1, space="PSUM") as ps_tr2,
            tc.tile_pool(name="ps_mm", bufs=2, space="PSUM") as ps_mm,
            tc.tile_pool(name="dram", bufs=1, space="DRAM") as dpool,
        ):
            # constants
            idx_sb = cpool.tile([128, S // 16], i16, tag="idx")
            nc.sync.dma_start(idx_sb[:], idx_d[:])
            dl_sb = cpool.tile([128, NCHT], f32, tag="dl")
            nc.sync.dma_start(dl_sb[:], dl_d[:])
            rec_sb = cpool.tile([TD, NT], f32, tag="rec")
            nc.sync.dma_start(rec_sb[:], rec_d[:])
            xT_sb = cpool.tile([D, NSH], f32, tag="xT")
            nc.sync.dma_start(xT_sb[:], xT_d[:])
            w1l_sb = cpool.tile([D, H], f32, tag="w1l")
            nc.sync.dma_start(w1l_sb[:], w1l_d[:])
            w1r_sb = cpool.tile([D, H], f32, tag="w1r")
            nc.sync.dma_start(w1r_sb[:], w1r_d[:])
            b1_sb = cpool.tile([H, 1], f32, tag="b1")
            nc.sync.dma_start(b1_sb[:], b1_d[:])
            w2l_sb = cpool.tile([H, O], f32, tag="w2l")
            nc.sync.dma_start(w2l_sb[:], w2l_d[:])
            w2r_sb = cpool.tile([H, O], f32, tag="w2r")
            nc.sync.dma_start(w2r_sb[:], w2r_d[:])
            b2_sb = cpool.tile([O, 1], f32, tag="b2")
            nc.sync.dma_start(b2_sb[:], b2_d[:])
            iota_sb = cpool.tile([128, TD], f32, tag="iota")
            nc.sync.dma_start(iota_sb[:], iota_d[:])
            id_sb = cpool.tile([TD, TD], f32, tag="ident")
            nc.sync.dma_start(id_sb[:], ident_d[:])

            h_sh = dpool.tile([NSH, D], f32, tag="h_sh")
            h_full = dpool.tile([NPAD, D], f32, tag="h_full")

            def layer(src_lo, src_hi, n_hi_rows, wl, wr, bias, dout, is_l1):
                """One SAGE layer; returns list of output-feature tiles."""
                aggT_tiles = []
                for b in batches:
                    n_lo, n_hi, n_all = b["n_lo"], b["n_hi"], b["n_all"]
                    msg = mpool.tile([128, n_all, D], f32, tag="msg")
                    c0 = b["slot0"] // 16
                    nc.gpsimd.dma_gather(
                        msg[:, 0:n_lo, :],
                        src_lo,
                        idx_sb[:, c0 : c0 + n_lo * 8],
                        n_lo * CH,
                        n_lo * CH,
                        D,
                        single_packet=False,
                    )
                    if n_hi > 0:
                        nc.gpsimd.dma_gather(
                            msg[:, n_lo:n_all, :],
                            src_hi,
                            idx_sb[:, c0 + n_lo * 8 : c0 + n_all * 8],
                            n_hi * CH,
                            n_hi * CH,
                            D,
                            single_packet=False,
                        )
                    # one-hot builds for the whole batch, SUB chunks at a time
                    ohs = []
                    for j0 in range(0, n_all, SUB):
                        kk = min(SUB, n_all - j0)
                        oh = ohpool.tile([128, SUB, TD], f32, tag="oh")
                        g0 = b["gchunk0"] + j0
                        dl_b = (
                            dl_sb[:, g0 : g0 + kk]
                            .rearrange("p (k o) -> p k o", o=1)
                            .to_broadcast((128, kk, TD))
                        )
                        io_b = (
                            iota_sb[:]
                            .rearrange("p (k t) -> p k t", k=1)
                            .to_broadcast((128, kk, TD))
                        )
                        nc.vector.tensor_tensor(
                            out=oh[:, 0:kk, :],
                            in0=dl_b,
                            in1=io_b,
                            op=mybir.AluOpType.is_equal,
                        )
                        ohs.append(oh)

                    for ti, (t, lo_c0, lo_n) in enumerate(b["lo_cols"]):
                        _, hi_c0, hi_n = b["hi_cols"][ti]
                        cols = list(range(lo_c0, lo_c0 + lo_n)) + list(
                            range(hi_c0, hi_c0 + hi_n)
                        )
                        ps = ps_sc.tile([TD, D], f32, tag="ps")
                        for ji, j in enumerate(cols):
                            nc.tensor.matmul(
                                out=ps[:],
                                lhsT=ohs[j // SUB][:, j % SUB, :],
                                rhs=msg[:, j, :],
                                start=(ji == 0),
                                stop=(ji == len(cols) - 1),
                            )
                        # normalize (mean) + evacuate
                        s1 = spool.tile([TD, D], f32, tag="s1")
                        nc.scalar.activation(
                            s1[:],
                            ps[:],
                            mybir.ActivationFunctionType.Copy,
                            scale=rec_sb[:, t : t + 1],
                        )
                        # transpose -> aggT half
                        if t % 2 == 0:
                            aggT = aggTpool.tile([TD, 128], f32, tag="aggT")
                            aggT_tiles.append(aggT)
                        aggT = aggT_tiles[t // 2]
                        pt = ps_tr.tile([TD, TD], f32, tag="pt")
                        nc.tensor.transpose(out=pt[:], in_=s1[:], identity=id_sb[:])
                        nc.vector.tensor_copy(
                            out=aggT[:, (t % 2) * TD : (t % 2) * TD + TD],
                            in_=pt[:],
                        )

                out_tiles = []
                for g in range(NSH // 128):
                    nsl = slice(g * 128, (g + 1) * 128)
                    Dout = H if is_l1 else O
                    ph = ps_mm.tile([Dout, 128], f32, tag="mm")
                    nc.tensor.matmul(
                        out=ph[:],
                        lhsT=wl[:],
                        rhs=aggT_tiles[g][:],
                        start=True,
                        stop=False,
                    )
                    rhs2 = xT_sb[:, nsl] if is_l1 else hT_tiles[g][:]
                    nc.tensor.matmul(
                        out=ph[:], lhsT=wr[:], rhs=rhs2, start=False, stop=True
                    )
                    if is_l1:
                        # ELU(z + b1) = relu(z+b1) + min(exp(z+b1) - 1, 0)
                        et = spool.tile([H, 128], f32, tag="et")
                        nc.scalar.activation(
                            et[:],
                            ph[:],
                            mybir.ActivationFunctionType.Exp,
                            bias=bias[:, 0:1],
                        )
                        hT = hTpool.tile([H, 128], f32, tag="hT")
                        nc.scalar.activation(
                            hT[:],
                            ph[:],
                            mybir.ActivationFunctionType.Relu,
                            bias=bias[:, 0:1],
                        )
                        nc.vector.tensor_scalar(
                            out=et[:],
                            in0=et[:],
                            scalar1=1.0,
                            scalar2=0.0,
                            op0=mybir.AluOpType.subtract,
                            op1=mybir.AluOpType.min,
                        )
                        nc.vector.tensor_tensor(
                            out=hT[:], in0=hT[:], in1=et[:], op=mybir.AluOpType.add
                        )
                        out_tiles.append(hT)
                        # node-major h for the gather of layer 2
                        pn = ps_tr2.tile([128, H], f32, tag="pn")
                        nc.tensor.transpose(out=pn[:], in_=hT[:], identity=id_sb[:])
                        hs = spool.tile([128, H], f32, tag="hs")
                        nc.vector.tensor_copy(out=hs[:], in_=pn[:])
                        if DEBUG_L1_ONLY:
                            nc.sync.dma_start(hout_d.ap()[nsl, :], hs[:])
                        else:
                            nc.sync.dma_start(h_sh[nsl, :], hs[:])
                    else:
                        ot = spool.tile([O, 128], f32, tag="ot")
                        nc.vector.tensor_scalar(
                            out=ot[:],
                            in0=ph[:],
                            scalar1=bias[:, 0:1],
                            scalar2=None,
                            op0=mybir.AluOpType.add,
                        )
                        nc.sync.dma_start(dout[:, nsl], ot[:])
                return out_tiles

            x_ap = x_d.ap()
            hT_tiles = layer(
                x_ap[0:HALFN, :],
                x_ap[HALFN:N, :],
                N - HALFN,
                w1l_sb,
                w1r_sb,
                b1_sb,
                None,
                True,
            )

            if not DEBUG_L1_ONLY:
                nc.gpsimd.collective_compute(
                    "AllGather",
                    mybir.AluOpType.bypass,
                    replica_groups=[list(range(NCORES))],
                    ins=[h_sh[:]],
                    outs=[h_full[:]],
                )

                if DEBUG_HFULL:
                    nc.sync.dma_start(hfull_d.ap()[:], h_full[:])

                hf = h_full[:]
                layer(
                    hf[0:HALFN, :],
                    hf[HALFN:NPAD, :],
                    NPAD - HALFN,
                    w2l_sb,
                    w2r_sb,
                    b2_sb,
                    outT_d.ap(),
                    False,
                )

    return nc


def _run(inputs, trace=False):
    x = np.ascontiguousarray(np.asarray(inputs["x"], dtype=np.float32))
    ei = np.asarray(inputs["edge_index"])
    W1l = np.ascontiguousarray(np.asarray(inputs["W1l"], np.float32))
    W1r = np.ascontiguousarray(np.asarray(inputs["W1r"], np.float32))
    b1 = np.asarray(inputs["b1"], np.float32).reshape(H, 1).copy()
    W2l = np.ascontiguousarray(np.asarray(inputs["W2l"], np.float32))
    W2r = np.ascontiguousarray(np.asarray(inputs["W2r"], np.float32))
    b2 = np.asarray(inputs["b2"], np.float32).reshape(O, 1).copy()

    struct, percore = _preprocess(ei)

    nc = bacc.Bacc(
        "TRN2",
        target_bir_lowering=False,
        debug=False,
        enable_asserts=False,
        num_devices=NCORES,
    )
    _build(nc, struct)
    nc.compile()

    iota = np.ascontiguousarray(
        np.tile(np.arange(TD, dtype=np.float32), (128, 1))
    )
    ident = np.eye(TD, dtype=np.float32)
    in_maps = []
    for k in range(NCORES):
        lo, hi = percore["shards"][k]
        xt = np.zeros((D, NSH), dtype=np.float32)
        xt[:, 0 : hi - lo] = x[lo:hi].T
        in_maps.append(
            dict(
                x=x,
                xT=np.ascontiguousarray(xt),
                idx=percore["idx"][k],
                dstloc=percore["dl"][k],
                rec=percore["rec"][k],
                w1l=W1l,
                w1r=W1r,
                b1=b1,
                w2l=W2l,
                w2r=W2r,
                b2=b2,
                iota=iota,
                ident=ident,
            )
        )

    res = run_bass_kernel_spmd(
        nc, in_maps, core_ids=list(range(NCORES)), trace=trace
    )
    outs = [res.results[k]["outT"] for k in range(NCORES)]
    full = np.concatenate([o.T for o in outs], axis=0)[:N]
    return np.ascontiguousarray(full.astype(np.float32)), res


def kernel(**inputs):
    out, _ = _run(inputs, trace=False)
    return out



# revision 4
# speedup vs baseline: 2.3039x; 2.3039x over previous
"""GraphSage (2-layer, mean aggr) on 8 trn2 NeuronCores — V2.

Scheme (dst-sharded edge-parallel, 4-queue SWDGE gather):
  - Nodes padded to 50176 = 8 * 6272; core c owns dst nodes [c*6272, (c+1)*6272).
  - dst tiles of TD=128 nodes (49 tiles/core); edges bucketed by (tile,
    src-quarter) where quarter q = src // 12544 (4 quarters = int16-safe
    gather bases AND 4 SWDGE queues running on disjoint Q7 core pairs).
  - Each (tile, quarter) list padded to a multiple of 128 (shared chunk
    structure = max count across cores; pad: src=0, dst_local=-1).
  - Gather x[src] rows (256B f32) with dma_gather, one call per
    (tile-batch, quarter) on queue q — 4 calls run concurrently.
  - Messages downcast f32->bf16 (ScalarE); one-hot built in bf16 on DVE
    (dl/iota bf16, is_equal); scatter via TensorE matmul with
    lhsT=msg [128e, 64f], rhs=onehot [128e, 128d] -> psum aggT [64f, 128d]
    (bf16 matmul = 1 cyc/col; aggT orientation avoids transposes).
  - Mean normalization folded into psum evacuation: aggT_sbuf =
    psum * rec_rep[:, tile] (DVE tensor_tensor, rec = 1/max(deg,1)).
  - Per tile: GEMM hT = W1l.T @ aggT + W1r.T @ xT (+b1), ELU =
    relu(z) + min(exp(z)-1, 0); hT kept bf16 for layer 2; h transposed
    node-major f32 -> DRAM, AllGather, layer 2 gathers from h_full.
"""

import sys

sys.path.insert(0, "/opt/trn_rl_repo")

import numpy as np

import concourse.bacc as bacc
import concourse.mybir as mybir
import concourse.tile as tile
from concourse.bass_utils import run_bass_kernel_spmd

N, E, D, H, O = 50000, 800000, 64, 64, 16
NCORES = 8
NSH = 6272                  # dst nodes per core
NPAD = NSH * NCORES         # 50176
TD = 128                    # dst tile size
NT = NSH // TD              # 49 dst tiles per core
CH = 128                    # edges per matmul chunk
NQ = 4                      # src quarters == SWDGE queues
QW = NPAD // NQ             # 12544 quarter width (int16-safe)
TB = 2                      # dst tiles per gather batch
SUB = 24                    # max chunks per one-hot build op

f32 = mybir.dt.float32
bf16 = mybir.dt.bfloat16
i16 = mybir.dt.int16

DEBUG_L1_ONLY = False


def _preprocess(edge_index):
    src = np.asarray(edge_index[0], dtype=np.int64)
    dst = np.asarray(edge_index[1], dtype=np.int64)
    core = dst // NSH
    ldst = dst - core * NSH
    tid = ldst // TD
    tloc = (ldst % TD).astype(np.float32)
    q = src // QW
    sq = (src - q * QW).astype(np.int16)

    key = ((core * NT + tid) * NQ + q)
    order = np.argsort(key, kind="stable")
    sq_s = sq[order]
    tloc_s = tloc[order]
    counts = np.bincount(key[order], minlength=NCORES * NT * NQ).reshape(
        NCORES, NT, NQ
    )
    starts = np.zeros(NCORES * NT * NQ + 1, dtype=np.int64)
    np.cumsum(counts.reshape(-1), out=starts[1:])

    cmax = counts.max(axis=0)                       # [NT, NQ]
    nch = ((cmax + CH - 1) // CH).astype(np.int64)  # [NT, NQ] chunks

    # batch structure: TB tiles; within batch chunks ordered quarter-major
    batches = []
    gchunk = 0
    for b0 in range(0, NT, TB):
        tids = list(range(b0, min(b0 + TB, NT)))
        qinfo = []      # per quarter: (arena_chunk_off, nchunks)
        tcols = {t: [] for t in tids}  # per tile: list of (arena_off, n)
        col = 0
        for qq in range(NQ):
            q0 = col
            for t in tids:
                n = int(nch[t, qq])
                if n:
                    tcols[t].append((col, n))
                col += n
            qinfo.append((q0, col - q0))
        batches.append(
            dict(tids=tids, qinfo=qinfo, tcols=tcols,
                 nchb=col, gchunk0=gchunk)
        )
        gchunk += col
    NCHT = gchunk
    S = NCHT * CH
    assert S % 16 == 0

    idx_wraps, dl_wraps, recs, xTs = [], [], [], []
    for k in range(NCORES):
        idx_flat = np.zeros(S, dtype=np.int16)
        dl_flat = np.full(S, -1.0, dtype=np.float32)
        for b in batches:
            for qq in range(NQ):
                coff = b["gchunk0"] + b["qinfo"][qq][0]
                for t in b["tids"]:
                    n = int(nch[t, qq])
                    if n == 0:
                        continue
                    ki = (k * NT + t) * NQ + qq
                    cnt = int(counts[k, t, qq])
                    s0 = coff * CH
                    e0 = int(starts[ki])
                    idx_flat[s0 : s0 + cnt] = sq_s[e0 : e0 + cnt]
                    dl_flat[s0 : s0 + cnt] = tloc_s[e0 : e0 + cnt]
                    coff += n
        iw = np.ascontiguousarray(
            np.tile(idx_flat.reshape(S // 16, 16).T, (8, 1))
        )
        idx_wraps.append(iw)
        dl_wraps.append(
            np.ascontiguousarray(
                dl_flat.reshape(NCHT, CH).T.astype(np.float32)
            ).astype(np.float32)
        )

        deg = np.bincount(ldst[core == k], minlength=NSH).astype(np.float32)
        rec = (1.0 / np.maximum(deg, 1.0)).astype(np.float32)
        recs.append(np.ascontiguousarray(np.tile(rec[None, :], (64, 1))))
        xTs.append((k * NSH, min((k + 1) * NSH, N)))

    struct = dict(batches=batches, S=S, NCHT=NCHT)
    percore = dict(idx=idx_wraps, dl=dl_wraps, rec=recs, shards=xTs)
    return struct, percore


def _build(nc, struct):
    S = struct["S"]
    NCHT = struct["NCHT"]
    batches = struct["batches"]

    x_d = nc.dram_tensor("x", [N, D], f32, kind="ExternalInput")
    xT_d = nc.dram_tensor("xT", [D, NSH], bf16, kind="ExternalInput")
    idx_d = nc.dram_tensor("idx", [128, S // 16], i16, kind="ExternalInput")
    dl_d = nc.dram_tensor("dstloc", [128, NCHT], bf16, kind="ExternalInput")
    rec_d = nc.dram_tensor("rec", [64, NSH], f32, kind="ExternalInput")
    w1l_d = nc.dram_tensor("w1l", [D, H], bf16, kind="ExternalInput")
    w1r_d = nc.dram_tensor("w1r", [D, H], bf16, kind="ExternalInput")
    b1_d = nc.dram_tensor("b1", [H, 1], f32, kind="ExternalInput")
    w2l_d = nc.dram_tensor("w2l", [H, O], bf16, kind="ExternalInput")
    w2r_d = nc.dram_tensor("w2r", [H, O], bf16, kind="ExternalInput")
    b2_d = nc.dram_tensor("b2", [O, 1], f32, kind="ExternalInput")
    iota_d = nc.dram_tensor("iota", [128, TD], bf16, kind="ExternalInput")
    ident_d = nc.dram_tensor("ident", [D, D], bf16, kind="ExternalInput")
    outT_d = nc.dram_tensor("outT", [O, NSH], f32, kind="ExternalOutput")
    hout_d = (
        nc.dram_tensor("hout", [NSH, D], f32, kind="ExternalOutput")
        if DEBUG_L1_ONLY
        else None
    )

    with tile.TileContext(nc, num_cores=NCORES) as tc:
        with (
            tc.tile_pool(name="const", bufs=1) as cpool,
            tc.tile_pool(name="msgq", bufs=3) as mqpool,
            tc.tile_pool(name="msgb", bufs=3) as mbpool,
            tc.tile_pool(name="oh", bufs=3) as ohpool,
            tc.tile_pool(name="aggT", bufs=4) as aggpool,
            tc.tile_pool(name="hT", bufs=NT) as hTpool,
            tc.tile_pool(name="small", bufs=4) as spool,
            tc.tile_pool(name="ps_sc", bufs=4, space="PSUM") as ps_sc,
            tc.tile_pool(name="ps_mm", bufs=2, space="PSUM") as ps_mm,
            tc.tile_pool(name="ps_tr", bufs=2, space="PSUM") as ps_tr,
            tc.tile_pool(name="dram", bufs=1, space="DRAM") as dpool,
        ):
            idx_sb = cpool.tile([128, S // 16], i16, tag="idx")
            nc.sync.dma_start(idx_sb[:], idx_d[:])
            dl_sb = cpool.tile([128, NCHT], bf16, tag="dl")
            nc.sync.dma_start(dl_sb[:], dl_d[:])
            rec_sb = cpool.tile([64, NSH], f32, tag="rec")
            nc.sync.dma_start(rec_sb[:], rec_d[:])
            xT_sb = cpool.tile([D, NSH], bf16, tag="xT")
            nc.sync.dma_start(xT_sb[:], xT_d[:])
            w1l_sb = cpool.tile([D, H], bf16, tag="w1l")
            nc.sync.dma_start(w1l_sb[:], w1l_d[:])
            w1r_sb = cpool.tile([D, H], bf16, tag="w1r")
            nc.sync.dma_start(w1r_sb[:], w1r_d[:])
            b1_sb = cpool.tile([H, 1], f32, tag="b1")
            nc.sync.dma_start(b1_sb[:], b1_d[:])
            w2l_sb = cpool.tile([H, O], bf16, tag="w2l")
            nc.sync.dma_start(w2l_sb[:], w2l_d[:])
            w2r_sb = cpool.tile([H, O], bf16, tag="w2r")
            nc.sync.dma_start(w2r_sb[:], w2r_d[:])
            b2_sb = cpool.tile([O, 1], f32, tag="b2")
            nc.sync.dma_start(b2_sb[:], b2_d[:])
            iota_sb = cpool.tile([128, TD], bf16, tag="iota")
            nc.sync.dma_start(iota_sb[:], iota_d[:])
            id_sb = cpool.tile([D, D], bf16, tag="ident")
            nc.sync.dma_start(id_sb[:], ident_d[:])

            h_sh = dpool.tile([NSH, D], f32, tag="h_sh")
            h_full = dpool.tile([NPAD, D], f32, tag="h_full")

            def layer(srcs, wl, wr, bias, is_l1):
                """One SAGE layer. srcs: list of NQ source APs (quarters)."""
                out_tiles = []
                for b in batches:
                    nchb = b["nchb"]
                    gc0 = b["gchunk0"]
                    # gather each quarter on its own queue
                    mq = []
                    for qq in range(NQ):
                        q0, qn = b["qinfo"][qq]
                        if qn == 0:
                            mq.append(None)
                            continue
                        m = mqpool.tile([128, qn, D], f32, tag=f"mq{qq}")
                        c0 = (gc0 + q0) * 8
                        nc.gpsimd.dma_gather(
                            m[:],
                            srcs[qq],
                            idx_sb[:, c0 : c0 + qn * 8],
                            qn * CH,
                            qn * CH,
                            D,
                            single_packet=False,
                            queue_num=qq,
                        )
                        mq.append(m)
                    # downcast to bf16 arena (quarter-major concat)
                    mb = mbpool.tile([128, nchb, D], bf16, tag="mb")
                    for qq in range(NQ):
                        q0, qn = b["qinfo"][qq]
                        if qn == 0:
                            continue
                        nc.scalar.activation(
                            mb[:, q0 : q0 + qn, :],
                            mq[qq][:],
                            mybir.ActivationFunctionType.Copy,
                        )
                    # one-hot build (bf16), SUB chunks at a time
                    ohs = []
                    for j0 in range(0, nchb, SUB):
                        kk = min(SUB, nchb - j0)
                        oh = ohpool.tile([128, SUB, TD], bf16, tag="oh")
                        dl_b = (
                            dl_sb[:, gc0 + j0 : gc0 + j0 + kk]
                            .rearrange("p (k o) -> p k o", o=1)
                            .to_broadcast((128, kk, TD))
                        )
                        io_b = (
                            iota_sb[:]
                            .rearrange("p (k t) -> p k t", k=1)
                            .to_broadcast((128, kk, TD))
                        )
                        nc.vector.tensor_tensor(
                            out=oh[:, 0:kk, :],
                            in0=dl_b,
                            in1=io_b,
                            op=mybir.AluOpType.is_equal,
                        )
                        ohs.append(oh)

                    for t in b["tids"]:
                        cols = []
                        for c0, n in b["tcols"][t]:
                            cols.extend(range(c0, c0 + n))
                        ps = ps_sc.tile([D, TD], f32, tag="ps")
                        for ji, j in enumerate(cols):
                            nc.tensor.matmul(
                                out=ps[:],
                                lhsT=mb[:, j, :],
                                rhs=ohs[j // SUB][:, j % SUB, :],
                                start=(ji == 0),
                                stop=(ji == len(cols) - 1),
                            )
                        nsl = slice(t * TD, (t + 1) * TD)
                        aggT = aggpool.tile([D, TD], bf16, tag="aggT")
                        nc.vector.tensor_tensor(
                            out=aggT[:],
                            in0=ps[:],
                            in1=rec_sb[:, nsl],
                            op=mybir.AluOpType.mult,
                        )
                        # dense GEMM for this node group
                        Dout = H if is_l1 else O
                        ph = ps_mm.tile([Dout, TD], f32, tag="mm")
                        nc.tensor.matmul(
                            out=ph[:], lhsT=wl[:], rhs=aggT[:],
                            start=True, stop=False,
                        )
                        rhs2 = xT_sb[:, nsl] if is_l1 else hT_tiles[t][:]
                        nc.tensor.matmul(
                            out=ph[:], lhsT=wr[:], rhs=rhs2,
                            start=False, stop=True,
                        )
                        if is_l1:
                            # ELU(z+b1) = relu(z+b1) + min(exp(z+b1)-1, 0)
                            et = spool.tile([H, TD], bf16, tag="et")
                            nc.scalar.activation(
                                et[:], ph[:],
                                mybir.ActivationFunctionType.Exp,
                                bias=bias[:, 0:1],
                            )
                            hT = hTpool.tile([H, TD], bf16, tag="hT")
                            nc.scalar.activation(
                                hT[:], ph[:],
                                mybir.ActivationFunctionType.Relu,
                                bias=bias[:, 0:1],
                            )
                            nc.vector.tensor_scalar(
                                out=et[:], in0=et[:],
                                scalar1=1.0, scalar2=0.0,
                                op0=mybir.AluOpType.subtract,
                                op1=mybir.AluOpType.min,
                            )
                            nc.vector.tensor_tensor(
                                out=hT[:], in0=hT[:], in1=et[:],
                                op=mybir.AluOpType.add,
                            )
                            out_tiles.append(hT)
                            # node-major f32 h for the layer-2 gather
                            pn = ps_tr.tile([TD, H], bf16, tag="pn")
                            nc.tensor.transpose(
                                out=pn[:], in_=hT[:], identity=id_sb[:]
                            )
                            hs = spool.tile([TD, H], f32, tag="hs")
                            nc.vector.tensor_copy(out=hs[:], in_=pn[:])
                            if DEBUG_L1_ONLY:
                                nc.sync.dma_start(hout_d.ap()[nsl, :], hs[:])
                            else:
                                nc.sync.dma_start(h_sh[nsl, :], hs[:])
                        else:
                            ot = spool.tile([O, TD], f32, tag="ot")
                            nc.vector.tensor_scalar(
                                out=ot[:], in0=ph[:],
                                scalar1=bias[:, 0:1], scalar2=None,
                                op0=mybir.AluOpType.add,
                            )
                            nc.sync.dma_start(outT_d.ap()[:, nsl], ot[:])
                return out_tiles

            x_ap = x_d.ap()
            xsrcs = [
                x_ap[qq * QW : min((qq + 1) * QW, N), :] for qq in range(NQ)
            ]
            hT_tiles = layer(xsrcs, w1l_sb, w1r_sb, b1_sb, True)

            if not DEBUG_L1_ONLY:
                nc.gpsimd.collective_compute(
                    "AllGather",
                    mybir.AluOpType.bypass,
                    replica_groups=[list(range(NCORES))],
                    ins=[h_sh[:]],
                    outs=[h_full[:]],
                )
                hf = h_full[:]
                hsrcs = [hf[qq * QW : (qq + 1) * QW, :] for qq in range(NQ)]
                layer(hsrcs, w2l_sb, w2r_sb, b2_sb, False)

    return nc


def _run(inputs, trace=False):
    x = np.ascontiguousarray(np.asarray(inputs["x"], dtype=np.float32))
    ei = np.asarray(inputs["edge_index"])
    W1l = np.asarray(inputs["W1l"], np.float32)
    W1r = np.asarray(inputs["W1r"], np.float32)
    b1 = np.asarray(inputs["b1"], np.float32).reshape(H, 1).copy()
    W2l = np.asarray(inputs["W2l"], np.float32)
    W2r = np.asarray(inputs["W2r"], np.float32)
    b2 = np.asarray(inputs["b2"], np.float32).reshape(O, 1).copy()

    struct, percore = _preprocess(ei)

    nc = bacc.Bacc(
        "TRN2",
        target_bir_lowering=False,
        debug=False,
        enable_asserts=False,
        num_devices=NCORES,
        num_swdge_queues=NQ,
    )
    _build(nc, struct)
    nc.compile()

    iota = np.ascontiguousarray(
        np.tile(np.arange(TD, dtype=np.float32), (128, 1))
    ).astype(np.float32)
    ident = np.eye(D, dtype=np.float32)

    def tobf(a):
        import jax.numpy as jnp

        return np.asarray(jnp.asarray(a, dtype=jnp.bfloat16))

    in_maps = []
    for k in range(NCORES):
        lo, hi = percore["shards"][k]
        xt = np.zeros((D, NSH), dtype=np.float32)
        xt[:, 0 : hi - lo] = x[lo:hi].T
        in_maps.append(
            dict(
                x=x,
                xT=tobf(xt),
                idx=percore["idx"][k],
                dstloc=tobf(percore["dl"][k]),
                rec=percore["rec"][k],
                w1l=tobf(W1l),
                w1r=tobf(W1r),
                b1=b1,
                w2l=tobf(W2l),
                w2r=tobf(W2r),
                b2=b2,
                iota=tobf(iota),
                ident=tobf(ident),
            )
        )

    res = run_bass_kernel_spmd(
        nc, in_maps, core_ids=list(range(NCORES)), trace=trace
    )
    if DEBUG_L1_ONLY:
        outs = [res.results[k]["hout"] for k in range(NCORES)]
        full = np.concatenate(outs, axis=0)[:N]
        return np.ascontiguousarray(full.astype(np.float32)), res
    outs = [res.results[k]["outT"] for k in range(NCORES)]
    full = np.concatenate([o.T for o in outs], axis=0)[:N]
    return np.ascontiguousarray(full.astype(np.float32)), res


def kernel(**inputs):
    out, _ = _run(inputs, trace=False)
    return out
